# revision 1
# baseline (speedup 1.0000x reference)
"""Multi-head attention block (B=8, S=1024, H=768, 12 heads x 64) on 8 TRN2 cores.

Sharding: pure data-parallel — one batch element per NeuronCore, no collectives.
Per core: QKV projections (f32r matmuls), attention with transposed-score layout
(softmax without max subtraction; rowsum via a ones-column appended to V),
residual + LayerNorm, all on one core.
"""

import sys

sys.path.insert(0, "/opt/trn_rl_repo")

import numpy as np
from contextlib import ExitStack

import concourse.bacc as bacc
import concourse.tile as tile
from concourse import mybir
from concourse import bass_utils
from concourse.masks import make_identity

AF = mybir.ActivationFunctionType
ALU = mybir.AluOpType
AX = mybir.AxisListType

F32 = mybir.dt.float32
F32R = mybir.dt.float32r
BF16 = mybir.dt.bfloat16

B, S, H, NH, DH = 8, 1024, 768, 12, 64
P = 128
HC = H // P   # 6 chunks of the feature dim
SC = S // P   # 8 chunks of the sequence dim
VW = NH * 65  # V storage width: 64 cols + 1 ones-col per head
EPS = 1e-6

_cache = {}


def _build(affine: bool, repeats: int = 1):
    nc = bacc.Bacc("TRN2", target_bir_lowering=False, debug=False)

    xT_d = nc.dram_tensor("xT", [H, S], F32R, kind="ExternalInput")
    xn_d = nc.dram_tensor("xn", [S, H], F32, kind="ExternalInput")
    wq_d = nc.dram_tensor("wq", [H, H], F32R, kind="ExternalInput")
    wk_d = nc.dram_tensor("wk", [H, H], F32R, kind="ExternalInput")
    wv_d = nc.dram_tensor("wv", [H, H], F32R, kind="ExternalInput")
    bq_d = nc.dram_tensor("bq", [H], F32, kind="ExternalInput")
    bk_d = nc.dram_tensor("bk", [H], F32, kind="ExternalInput")
    bv2_d = nc.dram_tensor("bv2", [H], F32, kind="ExternalInput")
    if affine:
        gam_d = nc.dram_tensor("gam", [H], F32, kind="ExternalInput")
        bet_d = nc.dram_tensor("bet", [H], F32, kind="ExternalInput")
    y_d = nc.dram_tensor("y", [S, H], F32, kind="ExternalOutput")

    dram = dict(xT_d=xT_d, xn_d=xn_d, wq_d=wq_d, wk_d=wk_d, wv_d=wv_d,
                bq_d=bq_d, bk_d=bk_d, bv2_d=bv2_d, y_d=y_d,
                gam_d=gam_d if affine else None,
                bet_d=bet_d if affine else None)
    with ExitStack() as stk:
        tc = stk.enter_context(tile.TileContext(nc))
        for rep in range(repeats):
            if rep:
                tc.strict_bb_all_engine_barrier()
            _emit_once(nc, tc, dram, affine, rep)
    nc.compile()
    return nc


def _emit_once(nc, tc, dram, affine, rep):
    xT_d, xn_d, y_d = dram["xT_d"], dram["xn_d"], dram["y_d"]
    wq_d, wk_d, wv_d = dram["wq_d"], dram["wk_d"], dram["wv_d"]
    bq_d, bk_d, bv2_d = dram["bq_d"], dram["bk_d"], dram["bv2_d"]
    gam_d, bet_d = dram["gam_d"], dram["bet_d"]
    with ExitStack() as stk:
        lp = stk.enter_context(tc.tile_pool(name=f"long{rep}", bufs=1))
        ap = stk.enter_context(tc.tile_pool(name=f"attn{rep}", bufs=1))
        ps = stk.enter_context(tc.tile_pool(name=f"ps{rep}", bufs=1, space="PSUM"))

        with tc.tile_pool(name=f"proj{rep}", bufs=1) as pp:
            # ---- phase A: loads ----
            xT = []
            for c in range(HC):
                t = pp.tile([P, S], F32R, tag=f"xt{c}")
                nc.sync.dma_start(t, xT_d[c * P:(c + 1) * P, :])
                xT.append(t)
            # weight chunks stream through shared per-chunk slots: the wq tile
            # is released after the Q projection and its slot reused for wk/wv
            W = {}
            for nm, d in (("q", wq_d), ("k", wk_d), ("v", wv_d)):
                W[nm] = []
                for c in range(HC):
                    t = pp.tile([P, H], F32R, tag=f"w{c}", bufs=2, name=f"w{nm}{c}")
                    nc.sync.dma_start(t, d[c * P:(c + 1) * P, :])
                    W[nm].append(t)
            bq_sb = lp.tile([P, HC], F32, tag="bq")
            nc.sync.dma_start(bq_sb, bq_d[:].rearrange("(c p) -> p c", p=P))
            bk_sb = lp.tile([P, HC], F32, tag="bk")
            nc.sync.dma_start(bk_sb, bk_d[:].rearrange("(c p) -> p c", p=P))

            # broadcast rows ([H] -> [128, H]) via a K=1 outer-product matmul
            ones1 = pp.tile([1, P], F32, tag="ones1")
            nc.vector.memset(ones1, 1.0)

            def bcast_row(d_ap, tag):
                row = pp.tile([1, H], F32, tag=f"{tag}row")
                nc.sync.dma_start(row, d_ap[:].rearrange("(o h) -> o h", o=1))
                pt = ps.tile([P, 1024], F32, tag="mm", bufs=1)
                for ns, nn in ((0, 512), (512, 256)):
                    nc.tensor.matmul(
                        pt[:, ns:ns + nn],
                        lhsT=ones1,
                        rhs=row[:, ns:ns + nn],
                        start=True, stop=True,
                    )
                bc = lp.tile([P, H], F32, tag=f"{tag}bc")
                nc.vector.tensor_copy(bc, pt[:, 0:H])
                return bc

            bv2bc = bcast_row(bv2_d, "bv2")
            if affine:
                gambc = bcast_row(gam_d, "gam")
                betbc = bcast_row(bet_d, "bet")

            # ---- phase B: projections ----
            # chunk 0 of Q/K first so head-0/1 scores (and ACT exp) start early
            QT = [None] * HC
            KT = [None] * HC

            def proj_qk_chunk(nm, b_sb, out_list, m):
                pt = ps.tile([P, 1024], F32, tag="mm", bufs=1,
                             name=f"p{nm}{m}")
                for ns in (0, 512):
                    for k in range(HC):
                        nc.tensor.matmul(
                            pt[:, ns:ns + 512],
                            lhsT=W[nm][k][:, m * P:(m + 1) * P],
                            rhs=xT[k][:, ns:ns + 512],
                            start=(k == 0), stop=(k == HC - 1),
                        )
                t = lp.tile([P, S], F32R, tag=f"{nm}t{m}", name=f"{nm}t{m}")
                nc.vector.tensor_scalar(
                    out=t, in0=pt, scalar1=b_sb[:, m:m + 1], scalar2=None,
                    op0=ALU.add,
                )
                out_list[m] = t

            Y = [lp.tile([P, H], F32, tag=f"y{m}", name=f"y{m}")
                 for m in range(SC)]
            expT = [[None] * SC for _ in range(NH)]

            def emit_scores_j(h, j):
                c, hp = divmod(h, 2)
                hp *= 64
                pt = ps.tile([P, 1024], F32, tag="mm", bufs=1, name=f"s{h}_{j}")
                for ns in (0, 512):
                    nc.tensor.matmul(
                        pt[:, ns:ns + 512],
                        lhsT=KT[c][hp:hp + 64, j * P:(j + 1) * P],
                        rhs=QT[c][hp:hp + 64, ns:ns + 512],
                        start=True, stop=True,
                    )
                et = ap.tile([P, S], BF16, tag="expt", bufs=12, name=f"e{h}_{j}")
                nc.scalar.activation(et, pt, AF.Exp, scale=1.0 / np.sqrt(DH))
                expT[h][j] = et

            for nm, b_sb, out_list in (("q", bq_sb, QT), ("k", bk_sb, KT)):
                for m in range(HC):
                    proj_qk_chunk(nm, b_sb, out_list, m)

            V = []
            for m in range(SC):
                pt = ps.tile([P, 1024], F32, tag="mm", bufs=1, name=f"pv{m}")
                for ns, nn in ((0, 512), (512, 256)):
                    for k in range(HC):
                        nc.tensor.matmul(
                            pt[:, ns:ns + nn],
                            lhsT=xT[k][:, m * P:(m + 1) * P],
                            rhs=W["v"][k][:, ns:ns + nn],
                            start=(k == 0), stop=(k == HC - 1),
                        )
                vt = lp.tile([P, VW], BF16, tag=f"v{m}", name=f"v{m}")
                v3 = vt.rearrange("p (h d) -> p h d", d=65)
                nc.vector.tensor_copy(
                    v3[:, :, 0:64],
                    pt[:, 0:H].rearrange("p (h d) -> p h d", d=64),
                )
                nc.vector.memset(v3[:, :, 64:65], 1.0)
                V.append(vt)

        # ---- phase C: attention pipeline ----
        for j in range(SC):
            emit_scores_j(0, j)

        # per head: 8 m-steps; each interleaves 2 score matmuls of head h+2
        # with the 8 accumulation matmuls of ctx(h, m), keeping ACT fed
        def emit_ctx_head(h, next_h):
            off = h * 65
            if next_h is not None:
                for j in range(SC):
                    emit_scores_j(next_h, j)
            for m in range(SC):
                pc = ps.tile([P, 65], F32, tag="ctx", bufs=2, name=f"c{h}_{m}")
                for j in range(SC):
                    nc.tensor.matmul(
                        pc[:, 0:65],
                        lhsT=expT[h][j][:, m * P:(m + 1) * P],
                        rhs=V[j][:, off:off + 65],
                        start=(j == 0), stop=(j == SC - 1),
                    )
                rinv = ap.tile([P, 1], F32, tag="rinv", bufs=6, name=f"r{h}_{m}")
                nc.vector.reciprocal(rinv, pc[:, 64:65])
                nc.vector.tensor_scalar(
                    out=Y[m][:, h * 64:(h + 1) * 64], in0=pc[:, 0:64],
                    scalar1=rinv, scalar2=2.0, op0=ALU.mult, op1=ALU.mult,
                )
            for j in range(SC):
                expT[h][j] = None

        for h in range(NH):
            emit_ctx_head(h, h + 1 if h + 1 < NH else None)


        # ---- phase D: residual + layernorm ----
        epsc = ap.tile([P, 1], F32, tag="epsc", bufs=1)
        nc.vector.memset(epsc, EPS)
        for m in range(SC):
            xs = ap.tile([P, H], F32, tag="xs", bufs=2)
            nc.sync.dma_start(xs, xn_d[m * P:(m + 1) * P, :])
            nc.vector.tensor_tensor(out=Y[m], in0=Y[m], in1=xs, op=ALU.add)
            nc.vector.tensor_tensor(out=Y[m], in0=Y[m], in1=bv2bc, op=ALU.add)
            sm = ap.tile([P, 1], F32, tag="sm", bufs=3)
            nc.vector.tensor_reduce(out=sm, in_=Y[m], axis=AX.X, op=ALU.add)
            nm_t = ap.tile([P, 1], F32, tag="nm", bufs=3)
            nc.vector.tensor_scalar(
                out=nm_t, in0=sm, scalar1=-1.0 / H, scalar2=None, op0=ALU.mult
            )
            nc.vector.tensor_scalar(
                out=Y[m], in0=Y[m], scalar1=nm_t, scalar2=None, op0=ALU.add
            )
            sq = ap.tile([P, H], F32, tag="sq", bufs=2)
            vs = ap.tile([P, 1], F32, tag="vs", bufs=3)
            nc.scalar.activation(sq, Y[m], AF.Square, accum_out=vs)
            sd = ap.tile([P, 1], F32, tag="sd", bufs=3)
            nc.scalar.activation(sd, vs, AF.Sqrt, scale=1.0 / H, bias=epsc[:, 0:1])
            rstd = ap.tile([P, 1], F32, tag="rstd", bufs=3)
            nc.vector.reciprocal(rstd, sd)
            nc.vector.tensor_scalar(
                out=Y[m], in0=Y[m], scalar1=rstd, scalar2=None, op0=ALU.mult
            )
            if affine:
                nc.vector.tensor_tensor(out=Y[m], in0=Y[m], in1=gambc, op=ALU.mult)
                nc.vector.tensor_tensor(out=Y[m], in0=Y[m], in1=betbc, op=ALU.add)
            nc.sync.dma_start(y_d[m * P:(m + 1) * P, :], Y[m])


def _get_nc(affine: bool):
    if affine not in _cache:
        _cache[affine] = _build(affine)
    return _cache[affine]


def _is_affine(inputs):
    gam = np.asarray(inputs["ln_gamma"], dtype=np.float32)
    bet = np.asarray(inputs["ln_beta"], dtype=np.float32)
    return not (np.all(gam == 1.0) and np.all(bet == 0.0))


def make_in_maps(inputs):
    x = np.ascontiguousarray(np.asarray(inputs["x"], dtype=np.float32))
    Wq = np.ascontiguousarray(np.asarray(inputs["Wq"], dtype=np.float32))
    Wk = np.ascontiguousarray(np.asarray(inputs["Wk"], dtype=np.float32))
    Wv = np.ascontiguousarray(np.asarray(inputs["Wv"], dtype=np.float32))
    bq = np.ascontiguousarray(np.asarray(inputs["bq"], dtype=np.float32))
    bk = np.ascontiguousarray(np.asarray(inputs["bk"], dtype=np.float32))
    bv = np.ascontiguousarray(np.asarray(inputs["bv"], dtype=np.float32))
    affine = _is_affine(inputs)

    in_maps = []
    for b in range(B):
        im = {
            "xT": np.ascontiguousarray(x[b].T),
            "xn": np.ascontiguousarray(x[b]),
            "wq": Wq, "wk": Wk, "wv": Wv,
            "bq": bq, "bk": bk, "bv2": (2.0 * bv).astype(np.float32),
        }
        if affine:
            im["gam"] = np.ascontiguousarray(
                np.asarray(inputs["ln_gamma"], dtype=np.float32))
            im["bet"] = np.ascontiguousarray(
                np.asarray(inputs["ln_beta"], dtype=np.float32))
        in_maps.append(im)
    return in_maps


def run(inputs, trace=False):
    nc = _get_nc(_is_affine(inputs))
    in_maps = make_in_maps(inputs)
    res = bass_utils.run_bass_kernel_spmd(
        nc, in_maps, core_ids=list(range(B)), trace=trace
    )
    out = np.stack([r["y"] for r in res.results], axis=0)
    return out, res


def kernel(**inputs) -> np.ndarray:
    out, _ = run(inputs, trace=False)
    return out

